# revision 1
# baseline (speedup 1.0000x reference)
"""Tensor-parallel GQA attention prefill (Llama-style) on one TRN2 chip.

Head-sharded across 8 NeuronCores: core c owns q-heads [4c, 4c+4) and
kv-head c.  x is replicated (pre-transposed on host), wq/wk/wv are
column-sharded, wo row-sharded; each core computes a partial output
[B*S, DIM] and the host sums the 8 partials.

Self-contained: shapes hardcoded for
  x[2,2048,4096] wq[4096,4096] wk/wv[1024,4096] wo[4096,4096]
  32 q heads / 8 kv heads / head_dim 128 / causal prefill (start_pos=0).
"""

import math

import numpy as np
import ml_dtypes

import concourse.bass as bass
import concourse.mybir as mybir
from concourse import bacc
from concourse.tile import TileContext
from concourse.bass_utils import run_bass_kernel_spmd
from concourse.masks import make_identity

BSZ, SEQ, DIM = 2, 2048, 4096
NH, NKV, HD = 32, 8, 128
NCORES = 8
HPC = NH // NCORES          # 4 q heads per core
BS = BSZ * SEQ              # 4096 flattened rows
NJ = BS // 512              # 8 s-chunks of 512
KT = DIM // 128             # 32 contraction tiles
SBLK = 4                    # 512-wide s-blocks per batch
BF16 = mybir.dt.bfloat16
F32 = mybir.dt.float32
NPBF16 = ml_dtypes.bfloat16
ALU = mybir.AluOpType
AF = mybir.ActivationFunctionType


def build_graph():
    nc = bacc.Bacc("TRN2", target_bir_lowering=False)
    xT = nc.declare_dram_parameter("xT", [DIM, BS], BF16, isOutput=False)
    wqT = nc.declare_dram_parameter("wqT", [DIM, HPC * HD], BF16, isOutput=False)
    wkT = nc.declare_dram_parameter("wkT", [DIM, HD], BF16, isOutput=False)
    wvT = nc.declare_dram_parameter("wvT", [DIM, HD], BF16, isOutput=False)
    woT = nc.declare_dram_parameter("woT", [HPC * HD, DIM], BF16, isOutput=False)
    cosT = nc.declare_dram_parameter("cosT", [HD // 2, SEQ], BF16, isOutput=False)
    sinT = nc.declare_dram_parameter("sinT", [HD // 2, SEQ], BF16, isOutput=False)
    m01 = nc.declare_dram_parameter("m01", [4, 128, 512], BF16, isOutput=False)
    out = nc.declare_dram_parameter("out", [BS, DIM], BF16, isOutput=True)
    den_dram = nc.dram_tensor("den_scratch", [8, 4, 512], F32)
    recip_dram = nc.dram_tensor("recip_scratch", [8, 4, 512], BF16)

    with TileContext(nc) as tc:
        with (
            tc.tile_pool(name="const", bufs=1) as const,
            tc.tile_pool(name="xtp", bufs=2) as xtp,
            tc.tile_pool(name="ropep", bufs=2) as ropep,
            tc.tile_pool(name="ptp", bufs=5) as ptp,
            tc.tile_pool(name="repp", bufs=2) as repp,
            tc.tile_pool(name="osb", bufs=2) as osb,
            tc.tile_pool(name="denp", bufs=2) as denp,
            tc.tile_pool(name="ps1", bufs=4, space="PSUM") as ps1,
            tc.tile_pool(name="ps2", bufs=4, space="PSUM") as ps2,
        ):
            # ---- resident constants / weights -------------------------------
            # wq/wk/wv are DMA'd per k-slice inside the j==0 loop so the
            # first matmuls start as soon as their slice lands.
            wq_sb = const.tile([128, KT, HPC * HD], BF16, tag="wq")
            wk_sb = const.tile([128, KT, HD], BF16, tag="wk")
            wv_sb = const.tile([128, KT, HD], BF16, tag="wv")
            cos_sb = const.tile([64, SEQ], BF16, tag="cos")
            nc.sync.dma_start(cos_sb[:], cosT[:])
            sin_sb = const.tile([64, SEQ], BF16, tag="sin")
            nc.sync.dma_start(sin_sb[:], sinT[:])
            # wo/m01 are first needed in the attention phase; loaded there.
            wo_sb = const.tile([128, HPC, DIM], BF16, tag="wo")
            m01_sb = const.tile([128, 4, 512], BF16, tag="m01")

            ones_sb = const.tile([128, 1], BF16, tag="ones")
            nc.gpsimd.memset(ones_sb[:], 1.0)
            ident = const.tile([128, 128], BF16, tag="ident")
            make_identity(nc, ident[:])

            # ---- resident activations ---------------------------------------
            qT_sb = const.tile([128, HPC, BS], BF16, tag="qT")    # per-head Q^T
            kT_sb = const.tile([128, BS], BF16, tag="kT")         # K^T (d, t)
            v_sb = const.tile([128, BS // 128, HD], BF16, tag="v")  # V (t, d) tiles
            attnT = const.tile([128, HPC, BS], BF16, tag="attnT")  # per-head out^T

            def rope_copy(psum, dst, soff):
                """psum [128,512] (evens-first layout) -> rotated bf16 dst."""
                te = ropep.tile([64, 512], BF16, tag="ropetmpe")
                to = ropep.tile([64, 512], BF16, tag="ropetmpo")
                nc.scalar.copy(te[:], psum[0:64])
                nc.vector.tensor_copy(to[:], psum[64:128])
                cs = cos_sb[:, soff:soff + 512]
                sn = sin_sb[:, soff:soff + 512]
                te = te[:]
                to = to[:]
                t1 = ropep.tile([64, 512], BF16, tag="t1")
                t2 = ropep.tile([64, 512], BF16, tag="t2")
                nc.vector.tensor_tensor(t1[:], te, cs, ALU.mult)
                nc.vector.tensor_tensor(t2[:], to, sn, ALU.mult)
                nc.vector.tensor_tensor(dst[0:64], t1[:], t2[:], ALU.subtract)
                t3 = ropep.tile([64, 512], BF16, tag="t1")
                t4 = ropep.tile([64, 512], BF16, tag="t2")
                nc.vector.tensor_tensor(t3[:], te, sn, ALU.mult)
                nc.vector.tensor_tensor(t4[:], to, cs, ALU.mult)
                nc.vector.tensor_tensor(dst[64:128], t3[:], t4[:], ALU.add)

            # ================= Phase 1: QKV projection =======================
            # single pass over xT per s-chunk: 4 Q accumulators in ps1,
            # K/V accumulators borrowed from ps2 (idle until attention).
            for j in range(NJ):
                soff = (j % SBLK) * 512      # within-batch s offset
                js = slice(j * 512, (j + 1) * 512)
                qps = [ps1.tile([128, 512], F32, tag="ps1", name=f"qps{j}_{c}") for c in range(HPC)]
                kp = ps2.tile([128, 512], F32, tag="ps2", name=f"kp{j}")
                vp = ps2.tile([128, 512], F32, tag="ps2", name=f"vp{j}")
                for kc in range(KT // 4):
                    if j == 0:
                        for k4 in range(4):
                            ks = slice((kc * 4 + k4) * 128, (kc * 4 + k4 + 1) * 128)
                            nc.sync.dma_start(wq_sb[:, kc * 4 + k4, :], wqT[ks, :])
                            nc.sync.dma_start(wk_sb[:, kc * 4 + k4, :], wkT[ks, :])
                            nc.sync.dma_start(wv_sb[:, kc * 4 + k4, :], wvT[ks, :])
                    xt = xtp.tile([128, 4, 512], BF16, tag="xt")
                    nc.sync.dma_start(
                        xt[:],
                        xT[kc * 512:(kc + 1) * 512, js].rearrange("(a p) m -> p a m", p=128))
                    for k4 in range(4):
                        k = kc * 4 + k4
                        for c in range(HPC):
                            nc.tensor.matmul(
                                qps[c][:], lhsT=wq_sb[:, k, c * 128:(c + 1) * 128],
                                rhs=xt[:, k4, :], start=(k == 0), stop=(k == KT - 1))
                        nc.tensor.matmul(kp[:], lhsT=wk_sb[:, k, :], rhs=xt[:, k4, :],
                                         start=(k == 0), stop=(k == KT - 1))
                        nc.tensor.matmul(vp[:], lhsT=wv_sb[:, k, :], rhs=xt[:, k4, :],
                                         start=(k == 0), stop=(k == KT - 1))
                for c in range(HPC):
                    rope_copy(qps[c], qT_sb[:, c, js], soff)
                rope_copy(kp, kT_sb[:, js], soff)
                # V^T chunk -> natural-layout V tiles via PE transpose
                vtmp = ropep.tile([128, 512], BF16, tag="vtmp")
                nc.scalar.copy(vtmp[:], vp[:])
                for sub in range(4):
                    tt = j * 4 + sub
                    pvt = ps1.tile([128, 512], BF16, tag="ps1")
                    with nc.allow_low_precision(reason="pure transpose, no accumulation"):
                        nc.tensor.transpose(
                            pvt[:, 0:128], vtmp[:, sub * 128:(sub + 1) * 128], ident[:])
                    nc.scalar.copy(v_sb[:, tt, :], pvt[:, 0:128])

            # ================= Phase 2+3: attention + out-proj ===============
            def emit_outproj(b, sj):
                for st in range(4):
                    s0 = (b * SEQ) + sj * 512 + st * 128
                    for n2 in range(4):
                        po = psb.tile([128, 1024], F32, tag="psb")
                        for half in range(2):
                            nn = n2 * 1024 + half * 512
                            for dt in range(HPC):
                                nc.tensor.matmul(
                                    po[:, half * 512:(half + 1) * 512],
                                    lhsT=attnT[:, dt, s0:s0 + 128],
                                    rhs=wo_sb[:, dt, nn:nn + 512],
                                    start=(dt == 0), stop=(dt == HPC - 1))
                        ob = osb.tile([128, 1024], BF16, tag="ob")
                        if n2 % 2 == 0:
                            nc.scalar.copy(ob[:], po[:])
                        else:
                            nc.vector.tensor_copy(ob[:], po[:])
                        nc.scalar.dma_start(
                            out[s0:s0 + 128, n2 * 1024:(n2 + 1) * 1024], ob[:])

            nc.sync.dma_start(m01_sb[:], m01.rearrange("k p s -> p k s"))
            nc.sync.dma_start(wo_sb[:], woT.rearrange("(a p) m -> p a m", p=128))

            # ================= Phase 2+3: attention + out-proj ===============
            def emit_outproj(b, sj, st_list=range(4)):
                for st in st_list:
                    s0 = (b * SEQ) + sj * 512 + st * 128
                    for n in range(8):
                        po = ps1.tile([128, 512], F32, tag="ps1")
                        for dt in range(HPC):
                            nc.tensor.matmul(
                                po[:], lhsT=attnT[:, dt, s0:s0 + 128],
                                rhs=wo_sb[:, dt, n * 512:(n + 1) * 512],
                                start=(dt == 0), stop=(dt == HPC - 1))
                        ob = osb.tile([128, 512], BF16, tag="ob")
                        if n % 2 == 0:
                            nc.scalar.copy(ob[:], po[:])
                        else:
                            nc.vector.tensor_copy(ob[:], po[:])
                        nc.sync.dma_start(out[s0:s0 + 128, n * 512:(n + 1) * 512], ob[:])

            prev = None
            for b in range(BSZ):
                for sj in range(SBLK):
                    sg = b * SEQ + sj * 512
                    nt = 4 * sj + 4                      # causal t-tiles
                    for h in range(HPC):
                        ppv = ps2.tile([128, 512], F32, tag="ps2")
                        pden = ps2.tile([128, 512], F32, tag="ps2")
                        for ti in range(nt):
                            tg = b * SEQ + ti * 128
                            psc = ps2.tile([128, 512], F32, tag="ps2")
                            diag = ti >= 4 * sj
                            if diag:                     # additive causal mask via PE
                                nc.tensor.matmul(psc[:], lhsT=ident[:],
                                                 rhs=m01_sb[:, ti - 4 * sj, :],
                                                 start=True, stop=False)
                            nc.tensor.matmul(psc[:], lhsT=kT_sb[:, tg:tg + 128],
                                             rhs=qT_sb[:, h, sg:sg + 512],
                                             start=not diag, stop=True)
                            pt = ptp.tile([128, 512], BF16, tag="pt")
                            nc.scalar.activation(pt[:], psc[:], AF.Exp)
                            nc.tensor.matmul(ppv[:], lhsT=v_sb[:, tg // 128, :],
                                             rhs=pt[:], start=(ti == 0),
                                             stop=(ti == nt - 1))
                            nc.tensor.matmul(pden[0:1, :], lhsT=ones_sb[:],
                                             rhs=pt[:], start=(ti == 0),
                                             stop=(ti == nt - 1))
                        bsj = b * SBLK + sj
                        dentmp = ropep.tile([1, 512], F32, tag="dentmp")
                        nc.scalar.copy(dentmp[:], pden[0:1, :])
                        nc.sync.dma_start(den_dram[bsj, h:h + 1, :], dentmp[:])
                        nc.scalar.copy(attnT[:, h, sg:sg + 512], ppv[:])
                    bsj = b * SBLK + sj
                    den_cur = denp.tile([32, 64], F32, tag="dencur")
                    nc.sync.dma_start(den_cur[:],
                                      den_dram[bsj].rearrange("h (a b) -> (h a) b", a=8))
                    rec_cur = denp.tile([32, 64], BF16, tag="reccur")
                    with nc.allow_low_precision(reason="softmax denom reciprocal in bf16"):
                        nc.vector.reciprocal(rec_cur[:], den_cur[:])
                    nc.sync.dma_start(recip_dram[bsj].rearrange("h (a b) -> (h a) b", a=8),
                                      rec_cur[:])
                    for h in range(HPC):
                        rep = repp.tile([128, 512], BF16, tag="rep")
                        nc.sync.dma_start(
                            rep[:],
                            recip_dram[bsj, h:h + 1, :].to_broadcast((128, 512)))
                        nc.vector.tensor_tensor(attnT[:, h, sg:sg + 512],
                                                attnT[:, h, sg:sg + 512],
                                                rep[:], ALU.mult)
                    if prev is not None:
                        emit_outproj(*prev)
                    prev = (b, sj)
            emit_outproj(*prev)
    nc.finalize()
    return nc


_GRAPH = None


def _get_graph():
    global _GRAPH
    if _GRAPH is None:
        _GRAPH = build_graph()
    return _GRAPH


def prepare_in_maps(x, wq, wk, wv, wo, freqs_cos, freqs_sin, mask, start_pos=0):
    x = np.asarray(x, np.float32)
    wq = np.asarray(wq, np.float32)
    wk = np.asarray(wk, np.float32)
    wv = np.asarray(wv, np.float32)
    wo = np.asarray(wo, np.float32)
    fc = np.asarray(freqs_cos, np.float32)
    fs = np.asarray(freqs_sin, np.float32)
    mask = np.asarray(mask, np.float32)

    # evens-first pair permutation (interleaved rope -> rotate-half form)
    perm = np.concatenate([np.arange(0, HD, 2), np.arange(1, HD, 2)])

    def permute_heads(w):
        wr = w.reshape(-1, HD, DIM)[:, perm, :]
        return wr.reshape(-1, DIM)

    wq_p = permute_heads(wq) * (1.0 / math.sqrt(HD))
    wk_p = permute_heads(wk)

    xT = np.ascontiguousarray(x.reshape(BS, DIM).T).astype(NPBF16)
    cosT = np.ascontiguousarray(fc.T).astype(NPBF16)
    sinT = np.ascontiguousarray(fs.T).astype(NPBF16)
    # additive causal mask tiles, transposed: m01[k][t, s] = mask[s, 128k + t]
    m01 = np.stack(
        [mask[0:512, 128 * k:128 * k + 128].T for k in range(4)]
    ).astype(NPBF16)

    in_maps = []
    for c in range(NCORES):
        qs = slice(c * HPC * HD, (c + 1) * HPC * HD)
        ks = slice(c * HD, (c + 1) * HD)
        in_maps.append({
            "xT": xT,
            "wqT": np.ascontiguousarray(wq_p[qs, :].T).astype(NPBF16),
            "wkT": np.ascontiguousarray(wk_p[ks, :].T).astype(NPBF16),
            "wvT": np.ascontiguousarray(wv[ks, :].T).astype(NPBF16),
            "woT": np.ascontiguousarray(wo[:, qs].T).astype(NPBF16),
            "cosT": cosT,
            "sinT": sinT,
            "m01": m01,
        })
    return in_maps


def combine_results(results):
    acc = results[0]["out"].astype(np.float64)
    for c in range(1, NCORES):
        acc = acc + results[c]["out"]
    return acc.astype(np.float32).reshape(BSZ, SEQ, DIM)


def run_spmd(in_maps, **kw):
    nc = _get_graph()
    return run_bass_kernel_spmd(nc, in_maps, list(range(NCORES)), **kw)


def kernel(x, wq, wk, wv, wo, freqs_cos, freqs_sin, mask, start_pos=0, **_):
    in_maps = prepare_in_maps(x, wq, wk, wv, wo, freqs_cos, freqs_sin, mask)
    res = run_spmd(in_maps)
    return combine_results(res.results)



# revision 2
# speedup vs baseline: 1.0617x; 1.0617x over previous
"""Tensor-parallel GQA attention prefill (Llama-style) on one TRN2 chip.

Head-sharded across 8 NeuronCores: core c owns q-heads [4c, 4c+4) and
kv-head c.  x is replicated (pre-transposed on host), wq/wk/wv are
column-sharded, wo row-sharded; each core computes a partial output
[B*S, DIM] and the host sums the 8 partials.

Self-contained: shapes hardcoded for
  x[2,2048,4096] wq[4096,4096] wk/wv[1024,4096] wo[4096,4096]
  32 q heads / 8 kv heads / head_dim 128 / causal prefill (start_pos=0).
"""

import math

import numpy as np
import ml_dtypes

import concourse.bass as bass
import concourse.mybir as mybir
from concourse import bacc
from concourse.tile import TileContext
from concourse.bass_utils import run_bass_kernel_spmd
from concourse.masks import make_identity

BSZ, SEQ, DIM = 2, 2048, 4096
NH, NKV, HD = 32, 8, 128
NCORES = 8
HPC = NH // NCORES          # 4 q heads per core
BS = BSZ * SEQ              # 4096 flattened rows
NJ = BS // 512              # 8 s-chunks of 512
KT = DIM // 128             # 32 contraction tiles
SBLK = 4                    # 512-wide s-blocks per batch
BF16 = mybir.dt.bfloat16
F32 = mybir.dt.float32
NPBF16 = ml_dtypes.bfloat16
ALU = mybir.AluOpType
AF = mybir.ActivationFunctionType


def build_graph():
    nc = bacc.Bacc("TRN2", target_bir_lowering=False)
    xT = nc.declare_dram_parameter("xT", [DIM, BS], BF16, isOutput=False)
    wqT = nc.declare_dram_parameter("wqT", [DIM, HPC * HD], BF16, isOutput=False)
    wkT = nc.declare_dram_parameter("wkT", [DIM, HD], BF16, isOutput=False)
    wvT = nc.declare_dram_parameter("wvT", [DIM, HD], BF16, isOutput=False)
    woT = nc.declare_dram_parameter("woT", [HPC * HD, DIM], BF16, isOutput=False)
    cosT = nc.declare_dram_parameter("cosT", [HD // 2, SEQ], BF16, isOutput=False)
    sinT = nc.declare_dram_parameter("sinT", [HD // 2, SEQ], BF16, isOutput=False)
    tri = nc.declare_dram_parameter("tri", [128, 128], BF16, isOutput=False)
    out = nc.declare_dram_parameter("out", [BS, DIM], BF16, isOutput=True)

    with TileContext(nc) as tc:
        with (
            tc.tile_pool(name="const", bufs=1) as const,
            tc.tile_pool(name="xtp", bufs=3) as xtp,
            tc.tile_pool(name="ropep", bufs=2) as ropep,
            tc.tile_pool(name="ptp", bufs=4) as ptp,
            tc.tile_pool(name="accp", bufs=2) as accp,
            tc.tile_pool(name="recp", bufs=2) as recp,
            tc.tile_pool(name="osb", bufs=3) as osb,
            tc.tile_pool(name="psA", bufs=4, space="PSUM") as psA,
            tc.tile_pool(name="psB", bufs=4, space="PSUM") as psB,
        ):
            # ---- resident constants / weights -------------------------------
            # wq/wk/wv are DMA'd per k-slice inside the j==0 loop so the
            # first matmuls start as soon as their slice lands.
            wq_sb = const.tile([128, KT, HPC * HD], BF16, tag="wq")
            wk_sb = const.tile([128, KT, HD], BF16, tag="wk")
            wv_sb = const.tile([128, KT, HD], BF16, tag="wv")
            cos_sb = const.tile([64, SEQ], BF16, tag="cos")
            nc.sync.dma_start(cos_sb[:], cosT[:])
            sin_sb = const.tile([64, SEQ], BF16, tag="sin")
            nc.sync.dma_start(sin_sb[:], sinT[:])
            tri_sb = const.tile([128, 128], BF16, tag="tri")
            nc.sync.dma_start(tri_sb[:], tri[:])
            # wo is first needed in the attention phase; loaded there.
            wo_sb = const.tile([128, HPC, DIM], BF16, tag="wo")

            ones_sb = const.tile([128, 1], BF16, tag="ones")
            nc.gpsimd.memset(ones_sb[:], 1.0)
            ident = const.tile([128, 128], BF16, tag="ident")
            make_identity(nc, ident[:])

            # ---- resident activations ---------------------------------------
            qT_sb = const.tile([128, HPC, BS], BF16, tag="qT")    # per-head Q^T
            kT_sb = const.tile([128, BS], BF16, tag="kT")         # K^T (d, t)
            v_sb = const.tile([128, BS // 128, HD], BF16, tag="v")  # V (t, d) tiles
            attnT = const.tile([128, HPC, BS], BF16, tag="attnT")  # per-head out^T

            def rope_copy(psum, dst, soff):
                """psum [128,512] (evens-first layout) -> rotated bf16 dst."""
                te = ropep.tile([64, 512], BF16, tag="ropetmpe")
                to = ropep.tile([64, 512], BF16, tag="ropetmpo")
                nc.scalar.copy(te[:], psum[0:64])
                nc.vector.tensor_copy(to[:], psum[64:128])
                cs = cos_sb[:, soff:soff + 512]
                sn = sin_sb[:, soff:soff + 512]
                te = te[:]
                to = to[:]
                t1 = ropep.tile([64, 512], BF16, tag="t1")
                t2 = ropep.tile([64, 512], BF16, tag="t2")
                nc.vector.tensor_tensor(t1[:], te, cs, ALU.mult)
                nc.vector.tensor_tensor(t2[:], to, sn, ALU.mult)
                nc.vector.tensor_tensor(dst[0:64], t1[:], t2[:], ALU.subtract)
                t3 = ropep.tile([64, 512], BF16, tag="t1")
                t4 = ropep.tile([64, 512], BF16, tag="t2")
                nc.vector.tensor_tensor(t3[:], te, sn, ALU.mult)
                nc.vector.tensor_tensor(t4[:], to, cs, ALU.mult)
                nc.vector.tensor_tensor(dst[64:128], t3[:], t4[:], ALU.add)

            # ================= Phase 1: QKV projection =======================
            # single pass over xT per s-chunk: 4 Q accumulators in psA,
            # K/V accumulators in psB.
            for j in range(NJ):
                soff = (j % SBLK) * 512      # within-batch s offset
                js = slice(j * 512, (j + 1) * 512)
                qps = [psA.tile([128, 512], F32, tag="psA", name=f"qps{j}_{c}") for c in range(HPC)]
                kp = psB.tile([128, 512], F32, tag="psB", name=f"kp{j}")
                vp = psB.tile([128, 512], F32, tag="psB", name=f"vp{j}")
                for kc in range(KT // 4):
                    if j == 0:
                        for k4 in range(4):
                            ks = slice((kc * 4 + k4) * 128, (kc * 4 + k4 + 1) * 128)
                            nc.sync.dma_start(wq_sb[:, kc * 4 + k4, :], wqT[ks, :])
                            nc.sync.dma_start(wk_sb[:, kc * 4 + k4, :], wkT[ks, :])
                            nc.sync.dma_start(wv_sb[:, kc * 4 + k4, :], wvT[ks, :])
                    xt = xtp.tile([128, 4, 512], BF16, tag="xt")
                    if j == 0:
                        # split so the first matmuls start after 128KB, not 512KB
                        for k4 in range(4):
                            nc.sync.dma_start(
                                xt[:, k4, :],
                                xT[(kc * 4 + k4) * 128:(kc * 4 + k4 + 1) * 128, js])
                    else:
                        nc.sync.dma_start(
                            xt[:],
                            xT[kc * 512:(kc + 1) * 512, js].rearrange("(a p) m -> p a m", p=128))
                    for k4 in range(4):
                        k = kc * 4 + k4
                        for c in range(HPC):
                            nc.tensor.matmul(
                                qps[c][:], lhsT=wq_sb[:, k, c * 128:(c + 1) * 128],
                                rhs=xt[:, k4, :], start=(k == 0), stop=(k == KT - 1))
                        nc.tensor.matmul(kp[:], lhsT=wk_sb[:, k, :], rhs=xt[:, k4, :],
                                         start=(k == 0), stop=(k == KT - 1))
                        nc.tensor.matmul(vp[:], lhsT=wv_sb[:, k, :], rhs=xt[:, k4, :],
                                         start=(k == 0), stop=(k == KT - 1))
                for c in range(HPC):
                    rope_copy(qps[c], qT_sb[:, c, js], soff)
                rope_copy(kp, kT_sb[:, js], soff)
                # V^T chunk -> natural-layout V tiles via PE transpose
                vtmp = ropep.tile([128, 512], BF16, tag="vtmp")
                nc.scalar.copy(vtmp[:], vp[:])
                for sub in range(4):
                    tt = j * 4 + sub
                    pvt = psB.tile([128, 512], BF16, tag="psB", name=f"pvt{j}_{sub}")
                    with nc.allow_low_precision(reason="pure transpose, no accumulation"):
                        nc.tensor.transpose(
                            pvt[:, 0:128], vtmp[:, sub * 128:(sub + 1) * 128], ident[:])
                    nc.scalar.copy(v_sb[:, tt, :], pvt[:, 0:128])

            nc.sync.dma_start(wo_sb[:], woT.rearrange("(a p) m -> p a m", p=128))

            # ================= Phase 2+3: attention + out-proj ===============
            # Out-proj of the previous block is interleaved at attention-tile
            # granularity so the PE never starves while ScalarE runs exp.
            def outproj_group(b, sj, g):
                st, n = g // 8, g % 8
                s0 = (b * SEQ) + sj * 512 + st * 128
                po = psA.tile([128, 512], F32, tag="psA", name=f"po{b}_{sj}_{g}")
                for dt in range(HPC):
                    nc.tensor.matmul(
                        po[:], lhsT=attnT[:, dt, s0:s0 + 128],
                        rhs=wo_sb[:, dt, n * 512:(n + 1) * 512],
                        start=(dt == 0), stop=(dt == HPC - 1))
                ob = osb.tile([128, 512], BF16, tag="ob")
                if n % 2 == 0:
                    nc.scalar.copy(ob[:], po[:])
                else:
                    nc.vector.tensor_copy(ob[:], po[:])
                nc.sync.dma_start(out[s0:s0 + 128, n * 512:(n + 1) * 512], ob[:])

            prev = None
            for b in range(BSZ):
                for sj in range(SBLK):
                    sg = b * SEQ + sj * 512
                    nt = 4 * sj + 4                      # causal t-tiles
                    emitted = 0
                    tidx = 0
                    for h in range(HPC):
                        ppv = psA.tile([128, 512], F32, tag="psA", name=f"ppv{b}{sj}{h}")
                        ptsum = accp.tile([128, 512], BF16, tag="ptsum",
                                          name=f"ptsum{b}{sj}{h}")
                        for ti in range(nt):
                            tg = b * SEQ + ti * 128
                            diag = ti >= 4 * sj
                            w0 = 128 * (ti - 4 * sj) if diag else 0
                            psc = psB.tile([128, 512], F32, tag="psB",
                                           name=f"psc{b}{sj}{h}{ti}")
                            nc.tensor.matmul(psc[:, w0:512],
                                             lhsT=kT_sb[:, tg:tg + 128],
                                             rhs=qT_sb[:, h, sg + w0:sg + 512],
                                             start=True, stop=not diag)
                            if diag:                     # additive causal mask via PE
                                nc.tensor.matmul(psc[:, w0:w0 + 128], lhsT=ident[:],
                                                 rhs=tri_sb[:],
                                                 start=False, stop=True)
                            if ti == 0:
                                pt = ptsum               # exp seeds the running sum
                            else:
                                pt = ptp.tile([128, 512], BF16, tag="pt")
                            nc.scalar.activation(pt[:, w0:512], psc[:, w0:512], AF.Exp)
                            if ti > 0:
                                nc.vector.tensor_tensor(ptsum[:, w0:512],
                                                        ptsum[:, w0:512],
                                                        pt[:, w0:512], ALU.add)
                            nc.tensor.matmul(ppv[:, w0:512],
                                             lhsT=v_sb[:, tg // 128, :],
                                             rhs=pt[:, w0:512], start=(ti == 0),
                                             stop=(ti == nt - 1))
                            tidx += 1
                            if prev is not None:
                                want = tidx * 32 // (HPC * nt)
                                while emitted < want:
                                    outproj_group(*prev, emitted)
                                    emitted += 1
                        # softmax denominator: one ones-matmul over the summed
                        # exp tiles, then reciprocal + broadcast, fused into
                        # the PSUM->SBUF copy of the attention output.
                        pden = psB.tile([1, 512], F32, tag="psB",
                                        name=f"pden{b}{sj}{h}")
                        nc.tensor.matmul(pden[:], lhsT=ones_sb[:], rhs=ptsum[:],
                                         start=True, stop=True)
                        rec = recp.tile([1, 512], BF16, tag="rec")
                        with nc.allow_low_precision(reason="softmax denom recip bf16"):
                            nc.vector.reciprocal(rec[:], pden[:])
                        rep = recp.tile([128, 512], BF16, tag="rep")
                        nc.gpsimd.partition_broadcast(rep[:], rec[:])
                        nc.vector.tensor_tensor(attnT[:, h, sg:sg + 512],
                                                ppv[:], rep[:], ALU.mult)
                    if prev is not None:
                        while emitted < 32:
                            outproj_group(*prev, emitted)
                            emitted += 1
                    prev = (b, sj)
            for g in range(32):
                outproj_group(*prev, g)
    nc.finalize()
    return nc


_GRAPH = None


def _get_graph():
    global _GRAPH
    if _GRAPH is None:
        _GRAPH = build_graph()
    return _GRAPH


def prepare_in_maps(x, wq, wk, wv, wo, freqs_cos, freqs_sin, mask, start_pos=0):
    x = np.asarray(x, np.float32)
    wq = np.asarray(wq, np.float32)
    wk = np.asarray(wk, np.float32)
    wv = np.asarray(wv, np.float32)
    wo = np.asarray(wo, np.float32)
    fc = np.asarray(freqs_cos, np.float32)
    fs = np.asarray(freqs_sin, np.float32)

    # evens-first pair permutation (interleaved rope -> rotate-half form)
    perm = np.concatenate([np.arange(0, HD, 2), np.arange(1, HD, 2)])

    def permute_heads(w):
        wr = w.reshape(-1, HD, DIM)[:, perm, :]
        return wr.reshape(-1, DIM)

    wq_p = permute_heads(wq) * (1.0 / math.sqrt(HD))
    wk_p = permute_heads(wk)

    xT = np.ascontiguousarray(x.reshape(BS, DIM).T).astype(NPBF16)
    cosT = np.ascontiguousarray(fc.T).astype(NPBF16)
    sinT = np.ascontiguousarray(fs.T).astype(NPBF16)
    # additive causal triangle for the in-tile diagonal: tri[t, c] = 0 if
    # c >= t else -1e9 (c = column within the 128-wide diagonal strip)
    tt, cc = np.meshgrid(np.arange(128), np.arange(128), indexing="ij")
    tri = np.where(cc >= tt, 0.0, -1e9).astype(NPBF16)

    in_maps = []
    for c in range(NCORES):
        qs = slice(c * HPC * HD, (c + 1) * HPC * HD)
        ks = slice(c * HD, (c + 1) * HD)
        in_maps.append({
            "xT": xT,
            "wqT": np.ascontiguousarray(wq_p[qs, :].T).astype(NPBF16),
            "wkT": np.ascontiguousarray(wk_p[ks, :].T).astype(NPBF16),
            "wvT": np.ascontiguousarray(wv[ks, :].T).astype(NPBF16),
            "woT": np.ascontiguousarray(wo[:, qs].T).astype(NPBF16),
            "cosT": cosT,
            "sinT": sinT,
            "tri": tri,
        })
    return in_maps


def combine_results(results):
    acc = results[0]["out"].astype(np.float64)
    for c in range(1, NCORES):
        acc = acc + results[c]["out"]
    return acc.astype(np.float32).reshape(BSZ, SEQ, DIM)


def run_spmd(in_maps, **kw):
    nc = _get_graph()
    return run_bass_kernel_spmd(nc, in_maps, list(range(NCORES)), **kw)


def kernel(x, wq, wk, wv, wo, freqs_cos, freqs_sin, mask, start_pos=0, **_):
    in_maps = prepare_in_maps(x, wq, wk, wv, wo, freqs_cos, freqs_sin, mask)
    res = run_spmd(in_maps)
    return combine_results(res.results)


# revision 5
# speedup vs baseline: 1.2187x; 1.1479x over previous
"""Tensor-parallel GQA attention prefill (Llama-style) on one TRN2 chip.

Head-sharded across 8 NeuronCores: core c owns q-heads [4c, 4c+4) and
kv-head c.  x is replicated (pre-transposed on host), wq/wk/wv are
column-sharded, wo row-sharded; each core computes a partial output
[B*S, DIM] and the host sums the 8 partials.

Self-contained: shapes hardcoded for
  x[2,2048,4096] wq[4096,4096] wk/wv[1024,4096] wo[4096,4096]
  32 q heads / 8 kv heads / head_dim 128 / causal prefill (start_pos=0).
"""

import math

import numpy as np
import ml_dtypes

import concourse.bass as bass
import concourse.mybir as mybir
from concourse import bacc
from concourse.tile import TileContext
from concourse.bass_utils import run_bass_kernel_spmd
from concourse.masks import make_identity

BSZ, SEQ, DIM = 2, 2048, 4096
NH, NKV, HD = 32, 8, 128
NCORES = 8
HPC = NH // NCORES          # 4 q heads per core
BS = BSZ * SEQ              # 4096 flattened rows
NJ = BS // 512              # 8 s-chunks of 512
KT = DIM // 128             # 32 contraction tiles
SBLK = 4                    # 512-wide s-blocks per batch
BF16 = mybir.dt.bfloat16
F32 = mybir.dt.float32
NPBF16 = ml_dtypes.bfloat16
ALU = mybir.AluOpType
AF = mybir.ActivationFunctionType


def build_graph():
    nc = bacc.Bacc("TRN2", target_bir_lowering=False)
    xT = nc.declare_dram_parameter("xT", [DIM, BS], BF16, isOutput=False)
    wqT = nc.declare_dram_parameter("wqT", [DIM, HPC * HD], BF16, isOutput=False)
    wkT = nc.declare_dram_parameter("wkT", [DIM, HD], BF16, isOutput=False)
    wvT = nc.declare_dram_parameter("wvT", [DIM, HD], BF16, isOutput=False)
    woT = nc.declare_dram_parameter("woT", [HPC * HD, DIM], BF16, isOutput=False)
    cosT = nc.declare_dram_parameter("cosT", [HD // 2, SEQ], BF16, isOutput=False)
    sinT = nc.declare_dram_parameter("sinT", [HD // 2, SEQ], BF16, isOutput=False)
    tri = nc.declare_dram_parameter("tri", [128, 128], BF16, isOutput=False)
    out = nc.declare_dram_parameter("out", [BS, DIM], BF16, isOutput=True)

    with TileContext(nc) as tc:
        with (
            tc.tile_pool(name="const", bufs=1) as const,
            tc.tile_pool(name="xtp", bufs=3) as xtp,
            tc.tile_pool(name="ropep", bufs=2) as ropep,
            tc.tile_pool(name="ptp", bufs=4) as ptp,
            tc.tile_pool(name="accp", bufs=2) as accp,
            tc.tile_pool(name="recp", bufs=2) as recp,
            tc.tile_pool(name="osb", bufs=3) as osb,
            tc.tile_pool(name="psA", bufs=4, space="PSUM") as psA,
            tc.tile_pool(name="psB", bufs=4, space="PSUM") as psB,
        ):
            # ---- resident constants / weights -------------------------------
            # wq/wk/wv are DMA'd per k-slice inside the j==0 loop so the
            # first matmuls start as soon as their slice lands.
            wq_sb = const.tile([128, KT, HPC * HD], BF16, tag="wq")
            wk_sb = const.tile([128, KT, HD], BF16, tag="wk")
            wv_sb = const.tile([128, KT, HD], BF16, tag="wv")
            cos_sb = const.tile([64, SEQ], BF16, tag="cos")
            nc.sync.dma_start(cos_sb[:], cosT[:])
            sin_sb = const.tile([64, SEQ], BF16, tag="sin")
            nc.sync.dma_start(sin_sb[:], sinT[:])
            tri_sb = const.tile([128, 128], BF16, tag="tri")
            nc.sync.dma_start(tri_sb[:], tri[:])
            # wo is first needed in the attention phase; loaded there.
            wo_sb = const.tile([128, HPC, DIM], BF16, tag="wo")

            ones_sb = const.tile([128, 1], BF16, tag="ones")
            nc.gpsimd.memset(ones_sb[:], 1.0)
            ident = const.tile([128, 128], BF16, tag="ident")
            make_identity(nc, ident[:])

            # ---- resident activations ---------------------------------------
            qT_sb = const.tile([128, HPC, BS], BF16, tag="qT")    # per-head Q^T
            kT_sb = const.tile([128, BS], BF16, tag="kT")         # K^T (d, t)
            v_sb = const.tile([128, BS // 128, HD], BF16, tag="v")  # V (t, d) tiles
            attnT = const.tile([128, HPC, BS], BF16, tag="attnT")  # per-head out^T

            def rope_copy(psum, dst, soff):
                """psum [128,512] (evens-first layout) -> rotated bf16 dst."""
                te = ropep.tile([64, 512], BF16, tag="ropetmpe")
                to = ropep.tile([64, 512], BF16, tag="ropetmpo")
                nc.scalar.copy(te[:], psum[0:64])
                nc.vector.tensor_copy(to[:], psum[64:128])
                cs = cos_sb[:, soff:soff + 512]
                sn = sin_sb[:, soff:soff + 512]
                te = te[:]
                to = to[:]
                t1 = ropep.tile([64, 512], BF16, tag="t1")
                t2 = ropep.tile([64, 512], BF16, tag="t2")
                nc.vector.tensor_tensor(t1[:], te, cs, ALU.mult)
                nc.vector.tensor_tensor(t2[:], to, sn, ALU.mult)
                nc.vector.tensor_tensor(dst[0:64], t1[:], t2[:], ALU.subtract)
                t3 = ropep.tile([64, 512], BF16, tag="t1")
                t4 = ropep.tile([64, 512], BF16, tag="t2")
                nc.vector.tensor_tensor(t3[:], te, sn, ALU.mult)
                nc.vector.tensor_tensor(t4[:], to, cs, ALU.mult)
                nc.vector.tensor_tensor(dst[64:128], t3[:], t4[:], ALU.add)

            # ================= Phase 1: QKV projection =======================
            # single pass over xT per s-chunk: 4 Q accumulators in psA,
            # K/V accumulators in psB.
            for j in range(NJ):
                soff = (j % SBLK) * 512      # within-batch s offset
                js = slice(j * 512, (j + 1) * 512)
                qps = [psA.tile([128, 512], F32, tag="psA", name=f"qps{j}_{c}") for c in range(HPC)]
                kp = psB.tile([128, 512], F32, tag="psB", name=f"kp{j}")
                vp = psB.tile([128, 512], F32, tag="psB", name=f"vp{j}")
                for kc in range(KT // 4):
                    xt = xtp.tile([128, 4, 512], BF16, tag="xt")
                    nc.sync.dma_start(
                        xt[:],
                        xT[kc * 512:(kc + 1) * 512, js].rearrange("(a p) m -> p a m", p=128))
                    if j == 0:
                        for k4 in range(4):
                            ks = slice((kc * 4 + k4) * 128, (kc * 4 + k4 + 1) * 128)
                            nc.sync.dma_start(wq_sb[:, kc * 4 + k4, :], wqT[ks, :])
                            nc.sync.dma_start(wk_sb[:, kc * 4 + k4, :], wkT[ks, :])
                            nc.sync.dma_start(wv_sb[:, kc * 4 + k4, :], wvT[ks, :])
                    for k4 in range(4):
                        k = kc * 4 + k4
                        for c in range(HPC):
                            nc.tensor.matmul(
                                qps[c][:], lhsT=wq_sb[:, k, c * 128:(c + 1) * 128],
                                rhs=xt[:, k4, :], start=(k == 0), stop=(k == KT - 1))
                        nc.tensor.matmul(kp[:], lhsT=wk_sb[:, k, :], rhs=xt[:, k4, :],
                                         start=(k == 0), stop=(k == KT - 1))
                        nc.tensor.matmul(vp[:], lhsT=wv_sb[:, k, :], rhs=xt[:, k4, :],
                                         start=(k == 0), stop=(k == KT - 1))
                for c in range(HPC):
                    rope_copy(qps[c], qT_sb[:, c, js], soff)
                rope_copy(kp, kT_sb[:, js], soff)
                # V^T chunk -> natural-layout V tiles via PE transpose.
                # Last chunk's copies go on DVE so the ScalarE queue is clear
                # for the first attention exp right at the phase boundary.
                last = j == NJ - 1
                vtmp = ropep.tile([128, 512], BF16, tag="vtmp")
                if last:
                    nc.vector.tensor_copy(vtmp[:], vp[:])
                else:
                    nc.scalar.copy(vtmp[:], vp[:])
                for sub in range(4):
                    tt = j * 4 + sub
                    pvt = psB.tile([128, 512], BF16, tag="psB", name=f"pvt{j}_{sub}")
                    with nc.allow_low_precision(reason="pure transpose, no accumulation"):
                        nc.tensor.transpose(
                            pvt[:, 0:128], vtmp[:, sub * 128:(sub + 1) * 128], ident[:])
                    if last:
                        nc.vector.tensor_copy(v_sb[:, tt, :], pvt[:, 0:128])
                    else:
                        nc.scalar.copy(v_sb[:, tt, :], pvt[:, 0:128])

            nc.sync.dma_start(wo_sb[:], woT.rearrange("(a p) m -> p a m", p=128))

            # ================= Phase 2+3: attention + out-proj ===============
            # Out-proj of the previous block is interleaved at attention-tile
            # granularity so the PE never starves while ScalarE runs exp.
            def outproj_group(b, sj, g):
                st, n = g // 8, g % 8
                s0 = (b * SEQ) + sj * 512 + st * 128
                po = psA.tile([128, 512], F32, tag="psA", name=f"po{b}_{sj}_{g}")
                for dt in range(HPC):
                    nc.tensor.matmul(
                        po[:], lhsT=attnT[:, dt, s0:s0 + 128],
                        rhs=wo_sb[:, dt, n * 512:(n + 1) * 512],
                        start=(dt == 0), stop=(dt == HPC - 1))
                ob = osb.tile([128, 512], BF16, tag="ob")
                if n % 2 == 0:
                    nc.scalar.copy(ob[:], po[:])
                else:
                    nc.vector.tensor_copy(ob[:], po[:])
                nc.sync.dma_start(out[s0:s0 + 128, n * 512:(n + 1) * 512], ob[:])

            prev = None
            for b in range(BSZ):
                for sj in range(SBLK):
                    sg = b * SEQ + sj * 512
                    nt = 4 * sj + 4                      # causal t-tiles
                    emitted = 0
                    tidx = 0
                    for h in range(HPC):
                        ppv = psA.tile([128, 512], F32, tag="psA", name=f"ppv{b}{sj}{h}")
                        ptsum = accp.tile([128, 512], BF16, tag="ptsum",
                                          name=f"ptsum{b}{sj}{h}")
                        for ti in range(nt):
                            tg = b * SEQ + ti * 128
                            diag = ti >= 4 * sj
                            w0 = 128 * (ti - 4 * sj) if diag else 0
                            psc = psB.tile([128, 512], F32, tag="psB",
                                           name=f"psc{b}{sj}{h}{ti}")
                            nc.tensor.matmul(psc[:, w0:512],
                                             lhsT=kT_sb[:, tg:tg + 128],
                                             rhs=qT_sb[:, h, sg + w0:sg + 512],
                                             start=True, stop=not diag)
                            if diag:                     # additive causal mask via PE
                                nc.tensor.matmul(psc[:, w0:w0 + 128], lhsT=ident[:],
                                                 rhs=tri_sb[:],
                                                 start=False, stop=True)
                            if ti == 0:
                                pt = ptsum               # exp seeds the running sum
                            else:
                                pt = ptp.tile([128, 512], BF16, tag="pt")
                            nc.scalar.activation(pt[:, w0:512], psc[:, w0:512], AF.Exp)
                            if ti > 0:
                                nc.vector.tensor_tensor(ptsum[:, w0:512],
                                                        ptsum[:, w0:512],
                                                        pt[:, w0:512], ALU.add)
                            nc.tensor.matmul(ppv[:, w0:512],
                                             lhsT=v_sb[:, tg // 128, :],
                                             rhs=pt[:, w0:512], start=(ti == 0),
                                             stop=(ti == nt - 1))
                            tidx += 1
                            if prev is not None:
                                want = tidx * 32 // (HPC * nt)
                                while emitted < want:
                                    outproj_group(*prev, emitted)
                                    emitted += 1
                        # softmax denominator: one ones-matmul over the summed
                        # exp tiles, then reciprocal + broadcast, fused into
                        # the PSUM->SBUF copy of the attention output.
                        pden = psB.tile([1, 512], F32, tag="psB",
                                        name=f"pden{b}{sj}{h}")
                        nc.tensor.matmul(pden[:], lhsT=ones_sb[:], rhs=ptsum[:],
                                         start=True, stop=True)
                        recf = recp.tile([1, 512], F32, tag="recf")
                        nc.vector.reciprocal_approx_fast(recf[:], pden[:])
                        rec = recp.tile([1, 512], BF16, tag="rec")
                        nc.scalar.copy(rec[:], recf[:])
                        rep = recp.tile([128, 512], BF16, tag="rep")
                        nc.gpsimd.partition_broadcast(rep[:], rec[:])
                        nc.vector.tensor_tensor(attnT[:, h, sg:sg + 512],
                                                ppv[:], rep[:], ALU.mult)
                    if prev is not None:
                        while emitted < 32:
                            outproj_group(*prev, emitted)
                            emitted += 1
                    prev = (b, sj)
            for g in range(32):
                outproj_group(*prev, g)
    nc.finalize()
    return nc


_GRAPH = None


def _get_graph():
    global _GRAPH
    if _GRAPH is None:
        _GRAPH = build_graph()
    return _GRAPH


def prepare_in_maps(x, wq, wk, wv, wo, freqs_cos, freqs_sin, mask, start_pos=0):
    x = np.asarray(x, np.float32)
    wq = np.asarray(wq, np.float32)
    wk = np.asarray(wk, np.float32)
    wv = np.asarray(wv, np.float32)
    wo = np.asarray(wo, np.float32)
    fc = np.asarray(freqs_cos, np.float32)
    fs = np.asarray(freqs_sin, np.float32)

    # evens-first pair permutation (interleaved rope -> rotate-half form)
    perm = np.concatenate([np.arange(0, HD, 2), np.arange(1, HD, 2)])

    def permute_heads(w):
        wr = w.reshape(-1, HD, DIM)[:, perm, :]
        return wr.reshape(-1, DIM)

    wq_p = permute_heads(wq) * (1.0 / math.sqrt(HD))
    wk_p = permute_heads(wk)

    xT = np.ascontiguousarray(x.reshape(BS, DIM).T).astype(NPBF16)
    cosT = np.ascontiguousarray(fc.T).astype(NPBF16)
    sinT = np.ascontiguousarray(fs.T).astype(NPBF16)
    # additive causal triangle for the in-tile diagonal: tri[t, c] = 0 if
    # c >= t else -1e9 (c = column within the 128-wide diagonal strip)
    tt, cc = np.meshgrid(np.arange(128), np.arange(128), indexing="ij")
    tri = np.where(cc >= tt, 0.0, -1e9).astype(NPBF16)

    in_maps = []
    for c in range(NCORES):
        qs = slice(c * HPC * HD, (c + 1) * HPC * HD)
        ks = slice(c * HD, (c + 1) * HD)
        in_maps.append({
            "xT": xT,
            "wqT": np.ascontiguousarray(wq_p[qs, :].T).astype(NPBF16),
            "wkT": np.ascontiguousarray(wk_p[ks, :].T).astype(NPBF16),
            "wvT": np.ascontiguousarray(wv[ks, :].T).astype(NPBF16),
            "woT": np.ascontiguousarray(wo[:, qs].T).astype(NPBF16),
            "cosT": cosT,
            "sinT": sinT,
            "tri": tri,
        })
    return in_maps


def combine_results(results):
    acc = results[0]["out"].astype(np.float64)
    for c in range(1, NCORES):
        acc = acc + results[c]["out"]
    return acc.astype(np.float32).reshape(BSZ, SEQ, DIM)


def run_spmd(in_maps, **kw):
    nc = _get_graph()
    return run_bass_kernel_spmd(nc, in_maps, list(range(NCORES)), **kw)


def kernel(x, wq, wk, wv, wo, freqs_cos, freqs_sin, mask, start_pos=0, **_):
    in_maps = prepare_in_maps(x, wq, wk, wv, wo, freqs_cos, freqs_sin, mask)
    res = run_spmd(in_maps)
    return combine_results(res.results)
